# revision 20
# baseline (speedup 1.0000x reference)
"""CBOW negative-sampling loss on 8 TRN2 NeuronCores.

Data-parallel: batch dim (16384) sharded 8 ways (2048 rows/core).

The gather (the memory-bound core of this problem) uses the bulk
InstDMAGatherAnt extended instruction (~900 rows per instruction,
rotated across the 4 SWDGE queues so all four Q7 core pairs generate
DMA descriptors in parallel) instead of per-row indirect DMAs, which
cost ~1us of serialized descriptor-generation per 128 rows.
dma_gather takes int16 indices (< 32768), but VOCAB=100000 — so the
host dedups and relabels each half-core's referenced rows into a
compacted table upload with static per-half slabs:

  - per core, per half (1024 batch rows), per table: the referenced
    vocab rows are uniqued (sorted) and relabeled 0..U-1; the table
    slab uploaded to the device is table[uniq] padded to a static cap
    (cap = the draw count, an absolute bound on U, ~20.5k < 32768).
  - gather indices are the relabeled ids, wrapped in dma_gather's
    [16, n/16] layout and replicated across all 128 partitions.

Per tile of 128 rows (one batch row per partition):
  - 3 dma_gathers: 20 context rows/partition -> ctx_g [128, 20, 128]
  - 3 dma_gathers: 20 negatives + 1 target   -> ng_g  [128, 21, 128]
  - ACT copies ctx_g to bf16 (exact-identity matmul inputs)
  - PE: 20 PSUM-accumulating bf16 identity matmuls -> ctx_sum (fp32)
  - DVE: broadcast-mult (in1 straight from PSUM) + reduce over EMB
    -> scores [128, 21]; clip to [-10, 10] in one chained min/max op
  - ACT Exp: negs (softplus(+s)) and target with scale=-1
    (softplus(-s) == -log_sigmoid(s)) into slices of exp_all
Final: one ACT Ln(1 + x) with accum_out over all 16*21 values (= sum
of softplus terms per partition), then a ones-vector matmul on the PE
reduces across partitions.  Host sums the 8 partials and divides by B.
"""

import os
import numpy as np

VOCAB, EMB = 100000, 128
B, C, N = 16384, 20, 20
NCORES = 8
RPC = B // NCORES  # 2048 rows per core
P = 128
TILES = RPC // P  # 16
N1 = N + 1  # negatives + target
HALVES = 2
TPH = TILES // HALVES  # tiles per half
CTX_CAP = TPH * P * C  # 20480 — absolute bound on unique rows per half
NG_CAP = TPH * P * N1  # 21504
CTX_W = P * C // 16  # 160 wrapped idx cols per tile
NG_W = P * N1 // 16  # 168

_compiled = None
last_results = None
import ml_dtypes as _mld

_IDENT = np.eye(P, dtype=_mld.bfloat16)


def _build(tiles=TILES, nqueues=4):
    import concourse.bacc as bacc
    import concourse.tile as tile
    from concourse import bass, library_config, mybir

    f32 = mybir.dt.float32
    bf16 = mybir.dt.bfloat16
    i16 = mybir.dt.int16
    AX = mybir.AxisListType
    OP = mybir.AluOpType
    AF = mybir.ActivationFunctionType

    nc = bacc.Bacc(
        "TRN2", target_bir_lowering=False, debug=False,
        num_swdge_queues=nqueues,
    )

    ctx_tab = nc.dram_tensor(
        "ctx_tab", [HALVES * CTX_CAP, EMB], f32, kind="ExternalInput"
    )
    out_tab = nc.dram_tensor(
        "out_tab", [HALVES * NG_CAP, EMB], f32, kind="ExternalInput"
    )
    ctx_widx = nc.dram_tensor(
        "ctx_widx", [P, tiles, CTX_W], i16, kind="ExternalInput"
    )
    ng_widx = nc.dram_tensor(
        "ng_widx", [P, tiles, NG_W], i16, kind="ExternalInput"
    )
    ident_in = nc.dram_tensor("ident", [P, P], bf16, kind="ExternalInput")
    partial = nc.dram_tensor("partial", [1, 1], f32, kind="ExternalOutput")

    with tile.TileContext(nc) as tc:
        with (
            tc.tile_pool(name="const", bufs=1) as cpool,
            tc.tile_pool(name="gather", bufs=6) as gpool,
            tc.tile_pool(name="work", bufs=3) as wpool,
            tc.tile_pool(name="psum", bufs=2, space=bass.MemorySpace.PSUM) as ppool,
        ):
            nc.gpsimd.load_library(library_config.mlp)

            ctx_widx_sb = cpool.tile([P, tiles, CTX_W], i16)
            nc.sync.dma_start(out=ctx_widx_sb[:], in_=ctx_widx[:])
            ng_widx_sb = cpool.tile([P, tiles, NG_W], i16)
            nc.sync.dma_start(out=ng_widx_sb[:], in_=ng_widx[:])

            ones = cpool.tile([P, 1], f32)
            nc.vector.memset(ones[:], 1.0)
            ident = cpool.tile([P, P], bf16)
            nc.sync.dma_start(out=ident[:], in_=ident_in[:])
            exp_all = cpool.tile([P, tiles, N1], f32)

            # dma_gather descriptor-ring capacity caps one call at ~1024
            # indices (HW-measured); split each tile's gather into
            # <=CHUNK-slot calls.
            CHUNK = int(os.environ.get('BASS_CHUNK', '7'))  # slots per call
            call_idx = [0]  # rotate SWDGE queues so the 4 Q7 core
            # pairs generate descriptors in parallel

            def gather_chunked(out_tile, tab_ap, widx_sb, t, cols):
                for c0 in range(0, cols, CHUNK):
                    c1 = min(c0 + CHUNK, cols)
                    n = P * (c1 - c0)
                    nc.gpsimd.dma_gather(
                        out_ap=out_tile[:, c0:c1, :],
                        in_ap=tab_ap,
                        idxs_ap=widx_sb[:, t, c0 * (P // 16) : c1 * (P // 16)],
                        num_idxs=n,
                        num_idxs_reg=n,
                        elem_size=EMB,
                        queue_num=call_idx[0] % nqueues,
                    )
                    call_idx[0] += 1

            for t in range(tiles):
                h = t // TPH
                ctx_g = gpool.tile([P, C, EMB], f32, tag="ctx_g")
                gather_chunked(
                    ctx_g, ctx_tab[h * CTX_CAP : (h + 1) * CTX_CAP, :],
                    ctx_widx_sb, t, C,
                )
                ng_g = gpool.tile([P, N1, EMB], f32, tag="ng_g")
                gather_chunked(
                    ng_g, out_tab[h * NG_CAP : (h + 1) * NG_CAP, :],
                    ng_widx_sb, t, N1,
                )

                # ctx sum via pairwise DVE tree (contiguous adds)
                s1 = wpool.tile([P, C // 2, EMB], f32, tag="s1")
                nc.vector.tensor_tensor(
                    out=s1[:], in0=ctx_g[:, 0:10, :], in1=ctx_g[:, 10:20, :],
                    op=OP.add,
                )
                s2 = wpool.tile([P, 5, EMB], f32, tag="s2")
                nc.vector.tensor_tensor(
                    out=s2[:], in0=s1[:, 0:5, :], in1=s1[:, 5:10, :],
                    op=OP.add,
                )
                s3 = wpool.tile([P, 2, EMB], f32, tag="s3")
                nc.vector.tensor_tensor(
                    out=s3[:], in0=s2[:, 0:2, :], in1=s2[:, 2:4, :],
                    op=OP.add,
                )
                s4 = wpool.tile([P, EMB], f32, tag="s4")
                nc.vector.tensor_tensor(
                    out=s4[:], in0=s3[:, 0, :], in1=s3[:, 1, :], op=OP.add,
                )
                ctx_sum = wpool.tile([P, EMB], f32, tag="ctx_sum")
                nc.vector.tensor_tensor(
                    out=ctx_sum[:], in0=s4[:], in1=s2[:, 4, :], op=OP.add,
                )

                prod = wpool.tile([P, N1, EMB], f32, tag="prod")
                nc.vector.tensor_tensor(
                    out=prod[:],
                    in0=ng_g[:],
                    in1=ctx_sum[:].unsqueeze(1).broadcast_to([P, N1, EMB]),
                    op=OP.mult,
                )
                scores = wpool.tile([P, N1], f32, tag="scores")
                nc.vector.tensor_reduce(
                    out=scores[:], in_=prod[:], axis=AX.X, op=OP.add
                )

                clipped = wpool.tile([P, N1], f32, tag="clipped")
                nc.vector.tensor_scalar(
                    out=clipped[:],
                    in0=scores[:],
                    scalar1=10.0,
                    scalar2=-10.0,
                    op0=OP.min,
                    op1=OP.max,
                )

                nc.scalar.activation(
                    out=exp_all[:, t, 0:N],
                    in_=clipped[:, 0:N],
                    func=AF.Exp,
                )
                nc.scalar.activation(
                    out=exp_all[:, t, N:N1],
                    in_=clipped[:, N:N1],
                    func=AF.Exp,
                    scale=-1.0,
                )

            # softplus = ln(1 + exp(x)); accum_out sums all tiles*N1
            # softplus terms per partition in the same pass.
            ln_all = wpool.tile([P, tiles * N1], f32, tag="ln_all")
            tot = wpool.tile([P, 1], f32, tag="tot")
            nc.scalar.activation(
                out=ln_all[:],
                in_=exp_all[:].rearrange("p t c -> p (t c)"),
                func=AF.Ln,
                bias=1.0,
                accum_out=tot[:],
            )
            ps = ppool.tile([1, 1], f32, tag="ps")
            nc.tensor.matmul(
                out=ps[:], lhsT=ones[:], rhs=tot[:], start=True, stop=True
            )
            res = wpool.tile([1, 1], f32, tag="res")
            nc.vector.tensor_copy(out=res[:], in_=ps[:])
            nc.sync.dma_start(out=partial[:], in_=res[:])

    nc.compile()
    return nc


def _wrap_idx(inv_blk):
    """[128, cols] relabeled per-(partition, slot) ids -> dma_gather's
    wrapped [128, P*cols/16] int16 layout (idx list position i = j*128+p,
    wrapped W[q, s] = L[s*16+q], replicated across the 8 groups of 16
    partitions)."""
    L = inv_blk.T.reshape(-1)  # L[j*128 + p]
    W = L.reshape(-1, 16).T  # [16, n/16]
    return np.tile(W, (8, 1)).astype(np.int16)


def _prep_core(ctxi, ngi, ctx_tab, out_tab):
    """Per-core host prep: dedup+relabel per half per table; build the
    compacted table slabs and wrapped index tiles."""
    ctx_tab_u = np.zeros((HALVES * CTX_CAP, EMB), np.float32)
    out_tab_u = np.zeros((HALVES * NG_CAP, EMB), np.float32)
    ctx_w = np.empty((P, TILES, CTX_W), np.int16)
    ng_w = np.empty((P, TILES, NG_W), np.int16)
    rph = TPH * P  # rows per half
    for h in range(HALVES):
        rows = slice(h * rph, (h + 1) * rph)
        for idx, cap, tab, tab_u, w, cols in (
            (ctxi[rows], CTX_CAP, ctx_tab, ctx_tab_u, ctx_w, C),
            (ngi[rows], NG_CAP, out_tab, out_tab_u, ng_w, N1),
        ):
            uniq, inv = np.unique(idx, return_inverse=True)
            assert len(uniq) <= cap
            tab_u[h * cap : h * cap + len(uniq)] = tab[uniq]
            inv = inv.reshape(rph, cols)
            for tt in range(TPH):
                t = h * TPH + tt
                w[:, t, :] = _wrap_idx(inv[tt * P : (tt + 1) * P])
    return ctx_tab_u, out_tab_u, ctx_w, ng_w


def _prep_in_maps(inputs):
    pos_target = np.asarray(inputs["pos_target"]).astype(np.int64).reshape(B)
    pos_contexts = (
        np.asarray(inputs["pos_contexts"]).astype(np.int64).reshape(B, C)
    )
    pos_negatives = (
        np.asarray(inputs["pos_negatives"]).astype(np.int64).reshape(B, N)
    )
    ctx_tab = np.ascontiguousarray(
        np.asarray(inputs["context_table"], dtype=np.float32)
    )
    out_tab = np.ascontiguousarray(
        np.asarray(inputs["output_table"], dtype=np.float32)
    )
    ng = np.concatenate([pos_negatives, pos_target[:, None]], axis=1)

    in_maps = []
    for i in range(NCORES):
        sl = slice(i * RPC, (i + 1) * RPC)
        ctx_tab_u, out_tab_u, ctx_w, ng_w = _prep_core(
            pos_contexts[sl], ng[sl], ctx_tab, out_tab
        )
        in_maps.append(
            {
                "ctx_tab": ctx_tab_u,
                "out_tab": out_tab_u,
                "ctx_widx": ctx_w,
                "ng_widx": ng_w,
                "ident": _IDENT,
            }
        )
    return in_maps


def kernel(**inputs) -> np.ndarray:
    global _compiled, last_results
    if _compiled is None:
        _compiled = _build()
    nc = _compiled

    from concourse.bass_utils import run_bass_kernel_spmd

    in_maps = _prep_in_maps(inputs)
    trace = os.environ.get("BASS_PROFILE", "") == "1"
    r = run_bass_kernel_spmd(nc, in_maps, list(range(NCORES)), trace=trace)
    last_results = r
    total = sum(float(r.results[i]["partial"][0, 0]) for i in range(NCORES))
    return np.asarray(total / B, dtype=np.float32)


# revision 21
# speedup vs baseline: 1.1546x; 1.1546x over previous
"""CBOW negative-sampling loss on 8 TRN2 NeuronCores.

Data-parallel: batch dim (16384) sharded 8 ways (2048 rows/core).

The gather (the memory-bound core of this problem) uses the bulk
InstDMAGatherAnt extended instruction (~900 rows per instruction,
rotated across the 4 SWDGE queues so all four Q7 core pairs generate
DMA descriptors in parallel) instead of per-row indirect DMAs, which
cost ~1us of serialized descriptor-generation per 128 rows.
dma_gather takes int16 indices (< 32768), but VOCAB=100000 — so the
host dedups and relabels each half-core's referenced rows into a
compacted table upload with static per-half slabs:

  - per core, per half (1024 batch rows), per table: the referenced
    vocab rows are uniqued (sorted) and relabeled 0..U-1; the table
    slab uploaded to the device is table[uniq] padded to a static cap
    (cap = the draw count, an absolute bound on U, ~20.5k < 32768).
  - gather indices are the relabeled ids, wrapped in dma_gather's
    [16, n/16] layout and replicated across all 128 partitions.

Per tile of 128 rows (one batch row per partition):
  - 3 dma_gathers: 20 context rows/partition -> ctx_g [128, 20, 128]
  - 3 dma_gathers: 20 negatives + 1 target   -> ng_g  [128, 21, 128]
  - ACT copies ctx_g to bf16 (exact-identity matmul inputs)
  - PE: 20 PSUM-accumulating bf16 identity matmuls -> ctx_sum (fp32)
  - DVE: broadcast-mult (in1 straight from PSUM) + reduce over EMB
    -> scores [128, 21]; clip to [-10, 10] in one chained min/max op
  - ACT Exp: negs (softplus(+s)) and target with scale=-1
    (softplus(-s) == -log_sigmoid(s)) into slices of exp_all
Final: one ACT Ln(1 + x) with accum_out over all 16*21 values (= sum
of softplus terms per partition), then a ones-vector matmul on the PE
reduces across partitions.  Host sums the 8 partials and divides by B.
"""

import os
import numpy as np

VOCAB, EMB = 100000, 128
B, C, N = 16384, 20, 20
NCORES = 8
RPC = B // NCORES  # 2048 rows per core
P = 128
TILES = RPC // P  # 16
N1 = N + 1  # negatives + target
HALVES = 2
TPH = TILES // HALVES  # tiles per half
CTX_CAP = TPH * P * C  # 20480 — absolute bound on unique rows per half
NG_CAP = TPH * P * N1  # 21504
CTX_W = P * C // 16  # 160 wrapped idx cols per tile
NG_W = P * N1 // 16  # 168

_compiled = None
last_results = None
import ml_dtypes as _mld

_IDENT = np.eye(P, dtype=_mld.bfloat16)


def _build(tiles=TILES, nqueues=4):
    import concourse.bacc as bacc
    import concourse.tile as tile
    from concourse import bass, library_config, mybir

    f32 = mybir.dt.float32
    bf16 = mybir.dt.bfloat16
    i16 = mybir.dt.int16
    AX = mybir.AxisListType
    OP = mybir.AluOpType
    AF = mybir.ActivationFunctionType

    nc = bacc.Bacc(
        "TRN2", target_bir_lowering=False, debug=False,
        num_swdge_queues=nqueues,
    )

    ctx_tab = nc.dram_tensor(
        "ctx_tab", [HALVES * CTX_CAP, EMB], f32, kind="ExternalInput"
    )
    out_tab = nc.dram_tensor(
        "out_tab", [HALVES * NG_CAP, EMB], f32, kind="ExternalInput"
    )
    ctx_widx = nc.dram_tensor(
        "ctx_widx", [P, tiles, CTX_W], i16, kind="ExternalInput"
    )
    ng_widx = nc.dram_tensor(
        "ng_widx", [P, tiles, NG_W], i16, kind="ExternalInput"
    )
    ident_in = nc.dram_tensor("ident", [P, P], bf16, kind="ExternalInput")
    partial = nc.dram_tensor("partial", [1, 1], f32, kind="ExternalOutput")

    with tile.TileContext(nc) as tc:
        with (
            tc.tile_pool(name="const", bufs=1) as cpool,
            tc.tile_pool(name="gather", bufs=6) as gpool,
            tc.tile_pool(name="work", bufs=3) as wpool,
            tc.tile_pool(name="psum", bufs=4, space=bass.MemorySpace.PSUM) as ppool,
        ):
            nc.gpsimd.load_library(library_config.mlp)

            ctx_widx_sb = cpool.tile([P, tiles, CTX_W], i16)
            nc.sync.dma_start(out=ctx_widx_sb[:], in_=ctx_widx[:])
            ng_widx_sb = cpool.tile([P, tiles, NG_W], i16)
            nc.sync.dma_start(out=ng_widx_sb[:], in_=ng_widx[:])

            ones = cpool.tile([P, 1], f32)
            nc.vector.memset(ones[:], 1.0)
            ident = cpool.tile([P, P], bf16)
            nc.sync.dma_start(out=ident[:], in_=ident_in[:])
            exp_all = cpool.tile([P, tiles, N1], f32)

            # dma_gather descriptor-ring capacity caps one call at ~1024
            # indices (HW-measured); split each tile's gather into
            # <=CHUNK-slot calls.
            CHUNK = int(os.environ.get('BASS_CHUNK', '7'))  # slots per call
            call_idx = [0]  # rotate SWDGE queues so the 4 Q7 core
            # pairs generate descriptors in parallel

            def gather_chunked(out_tile, tab_ap, widx_sb, t, cols):
                for c0 in range(0, cols, CHUNK):
                    c1 = min(c0 + CHUNK, cols)
                    n = P * (c1 - c0)
                    nc.gpsimd.dma_gather(
                        out_ap=out_tile[:, c0:c1, :],
                        in_ap=tab_ap,
                        idxs_ap=widx_sb[:, t, c0 * (P // 16) : c1 * (P // 16)],
                        num_idxs=n,
                        num_idxs_reg=n,
                        elem_size=EMB,
                        queue_num=call_idx[0] % nqueues,
                    )
                    call_idx[0] += 1

            for t in range(tiles):
                h = t // TPH
                ctx_g = gpool.tile([P, C, EMB], f32, tag="ctx_g")
                gather_chunked(
                    ctx_g, ctx_tab[h * CTX_CAP : (h + 1) * CTX_CAP, :],
                    ctx_widx_sb, t, C,
                )
                ng_g = gpool.tile([P, N1, EMB], f32, tag="ng_g")
                gather_chunked(
                    ng_g, out_tab[h * NG_CAP : (h + 1) * NG_CAP, :],
                    ng_widx_sb, t, N1,
                )

                ctx_bf = wpool.tile([P, C, EMB], bf16, tag="ctx_bf")
                nc.scalar.activation(
                    out=ctx_bf[:], in_=ctx_g[:], func=AF.Copy
                )
                ctx_sum = ppool.tile([P, EMB], f32, tag="ctx_sum")
                for c in range(C):
                    nc.tensor.matmul(
                        out=ctx_sum[:],
                        lhsT=ident[:],
                        rhs=ctx_bf[:, c, :],
                        start=(c == 0),
                        stop=(c == C - 1),
                    )

                prod = wpool.tile([P, N1, EMB], f32, tag="prod")
                nc.vector.tensor_tensor(
                    out=prod[:],
                    in0=ng_g[:],
                    in1=ctx_sum[:].unsqueeze(1).broadcast_to([P, N1, EMB]),
                    op=OP.mult,
                )
                scores = wpool.tile([P, N1], f32, tag="scores")
                nc.vector.tensor_reduce(
                    out=scores[:], in_=prod[:], axis=AX.X, op=OP.add
                )

                clipped = wpool.tile([P, N1], f32, tag="clipped")
                nc.vector.tensor_scalar(
                    out=clipped[:],
                    in0=scores[:],
                    scalar1=10.0,
                    scalar2=-10.0,
                    op0=OP.min,
                    op1=OP.max,
                )

                nc.scalar.activation(
                    out=exp_all[:, t, 0:N],
                    in_=clipped[:, 0:N],
                    func=AF.Exp,
                )
                nc.scalar.activation(
                    out=exp_all[:, t, N:N1],
                    in_=clipped[:, N:N1],
                    func=AF.Exp,
                    scale=-1.0,
                )

            # softplus = ln(1 + exp(x)); accum_out sums all tiles*N1
            # softplus terms per partition in the same pass.
            ln_all = wpool.tile([P, tiles * N1], f32, tag="ln_all")
            tot = wpool.tile([P, 1], f32, tag="tot")
            nc.scalar.activation(
                out=ln_all[:],
                in_=exp_all[:].rearrange("p t c -> p (t c)"),
                func=AF.Ln,
                bias=1.0,
                accum_out=tot[:],
            )
            ps = ppool.tile([1, 1], f32, tag="ps")
            nc.tensor.matmul(
                out=ps[:], lhsT=ones[:], rhs=tot[:], start=True, stop=True
            )
            res = wpool.tile([1, 1], f32, tag="res")
            nc.vector.tensor_copy(out=res[:], in_=ps[:])
            nc.sync.dma_start(out=partial[:], in_=res[:])

    nc.compile()
    return nc


def _wrap_idx(inv_blk):
    """[128, cols] relabeled per-(partition, slot) ids -> dma_gather's
    wrapped [128, P*cols/16] int16 layout (idx list position i = j*128+p,
    wrapped W[q, s] = L[s*16+q], replicated across the 8 groups of 16
    partitions)."""
    L = inv_blk.T.reshape(-1)  # L[j*128 + p]
    W = L.reshape(-1, 16).T  # [16, n/16]
    return np.tile(W, (8, 1)).astype(np.int16)


def _prep_core(ctxi, ngi, ctx_tab, out_tab):
    """Per-core host prep: dedup+relabel per half per table; build the
    compacted table slabs and wrapped index tiles."""
    ctx_tab_u = np.zeros((HALVES * CTX_CAP, EMB), np.float32)
    out_tab_u = np.zeros((HALVES * NG_CAP, EMB), np.float32)
    ctx_w = np.empty((P, TILES, CTX_W), np.int16)
    ng_w = np.empty((P, TILES, NG_W), np.int16)
    rph = TPH * P  # rows per half
    for h in range(HALVES):
        rows = slice(h * rph, (h + 1) * rph)
        for idx, cap, tab, tab_u, w, cols in (
            (ctxi[rows], CTX_CAP, ctx_tab, ctx_tab_u, ctx_w, C),
            (ngi[rows], NG_CAP, out_tab, out_tab_u, ng_w, N1),
        ):
            uniq, inv = np.unique(idx, return_inverse=True)
            assert len(uniq) <= cap
            tab_u[h * cap : h * cap + len(uniq)] = tab[uniq]
            inv = inv.reshape(rph, cols)
            for tt in range(TPH):
                t = h * TPH + tt
                w[:, t, :] = _wrap_idx(inv[tt * P : (tt + 1) * P])
    return ctx_tab_u, out_tab_u, ctx_w, ng_w


def _prep_in_maps(inputs):
    pos_target = np.asarray(inputs["pos_target"]).astype(np.int64).reshape(B)
    pos_contexts = (
        np.asarray(inputs["pos_contexts"]).astype(np.int64).reshape(B, C)
    )
    pos_negatives = (
        np.asarray(inputs["pos_negatives"]).astype(np.int64).reshape(B, N)
    )
    ctx_tab = np.ascontiguousarray(
        np.asarray(inputs["context_table"], dtype=np.float32)
    )
    out_tab = np.ascontiguousarray(
        np.asarray(inputs["output_table"], dtype=np.float32)
    )
    ng = np.concatenate([pos_negatives, pos_target[:, None]], axis=1)

    in_maps = []
    for i in range(NCORES):
        sl = slice(i * RPC, (i + 1) * RPC)
        ctx_tab_u, out_tab_u, ctx_w, ng_w = _prep_core(
            pos_contexts[sl], ng[sl], ctx_tab, out_tab
        )
        in_maps.append(
            {
                "ctx_tab": ctx_tab_u,
                "out_tab": out_tab_u,
                "ctx_widx": ctx_w,
                "ng_widx": ng_w,
                "ident": _IDENT,
            }
        )
    return in_maps


def kernel(**inputs) -> np.ndarray:
    global _compiled, last_results
    if _compiled is None:
        _compiled = _build()
    nc = _compiled

    from concourse.bass_utils import run_bass_kernel_spmd

    in_maps = _prep_in_maps(inputs)
    trace = os.environ.get("BASS_PROFILE", "") == "1"
    r = run_bass_kernel_spmd(nc, in_maps, list(range(NCORES)), trace=trace)
    last_results = r
    total = sum(float(r.results[i]["partial"][0, 0]) for i in range(NCORES))
    return np.asarray(total / B, dtype=np.float32)
